# revision 16
# baseline (speedup 1.0000x reference)
"""Trainium2 Bass kernel for nn_DNDecoder (GNN edge-MLP decoder).

out[e] = W2 @ LeakyReLU(W1 @ [z[row_e]; z[col_e]] + b1) + b2   for 1.6M edges.

Strategy (8 NeuronCores, edges sharded data-parallel):
  - z is cast to fp16 and replicated on every core; per-edge node features are
    fetched with the GPSIMD transposed dma_gather, which lands z rows as
    *columns* [h=128 partitions, e free] directly in SBUF — the layout matmul
    needs, no on-chip transposes.
  - dma_gather indices are int16 (<32768), so nodes are split into 4 windows
    of 32768; each core's edges are sorted into the 16 (row-window,
    col-window) groups. Group capacities are shared across cores so one SPMD
    program serves all 8 cores. Output is un-permuted on the host.
  - Both gathers ride SWDGE queue 3 (nqueues=4): concurrent use of multiple
    SWDGE queues corrupts data / crashes (shared transpose-xbar state), but a
    higher queue index activates more Q7 desc-gen channels, and a 64KB
    descriptor ring (4096 descs/engine vs default 1024) lets one 8192-idx
    call's desc-gen overlap the previous call's drain.
  - Per 512-edge tile: PSUM U = W1aT.T@zr + W1bT.T@zc (2 fp16 matmuls),
    one ScalarE activation does bias + LeakyReLU + fp16 cast, then a third
    matmul with W2 embedded at the tile's column of a [128,32] stationary
    accumulates 128 tiles of final dot products into one PSUM bank
    ([128 tiles, 512 lanes]), copied out once per 65536 edges.
"""

import sys

for p in ("/opt/trn_rl_repo", "/opt/pypackages"):
    if p not in sys.path:
        sys.path.append(p)

import numpy as np

N_NODES = 100000
H = 128
E_TOTAL = 1600000
N_CORES = 8
EDGES_PER_CORE = E_TOTAL // N_CORES
BWIN = 32768          # index window (int16 gather limit)
NB = 4                # node windows
TILE = 512            # edges per matmul tile (one PSUM bank)
SUP = 128             # tiles per output supertile (one PSUM bank of results)
GATHER_N = 8192       # max edges per dma_gather call
SORT_KEY = "gr"       # within-group edge order: "g" none, "gr" by row, "gc" by col


def _plan(row, col):
    """Shared-structure plan across cores.

    Returns (caps, segments, NTILES, NSUP) and per-core
    (order, local_r, local_c) where order[i] = original edge position of the
    i-th edge in group-sorted order.
    """
    per_core = []
    sizes = np.zeros((N_CORES, NB * NB), np.int64)
    for c in range(N_CORES):
        r = row[c * EDGES_PER_CORE:(c + 1) * EDGES_PER_CORE]
        cc = col[c * EDGES_PER_CORE:(c + 1) * EDGES_PER_CORE]
        g = (r >> 15) * NB + (cc >> 15)
        # sort by (group, row): ascending row idx within a group gives each
        # DMA engine a quasi-sequential HBM read stream for the zr gather
        if SORT_KEY == "gr":
            key = (g.astype(np.int64) << 17) | r
        elif SORT_KEY == "gc":
            key = (g.astype(np.int64) << 17) | cc
        else:
            key = g
        order = np.argsort(key, kind="stable")
        gs = g[order]
        sizes[c] = np.bincount(g, minlength=NB * NB)
        per_core.append((order, r[order], cc[order], gs))
    caps = ((sizes.max(axis=0) + TILE - 1) // TILE) * TILE
    segments = []  # (group, n, tot_off)
    off = 0
    for g in range(NB * NB):
        rem = int(caps[g])
        while rem > 0:
            n = min(GATHER_N, rem)
            segments.append((g, n, off))
            off += n
            rem -= n
    tot = off
    ntiles = tot // TILE
    nsup = (ntiles + SUP - 1) // SUP
    return caps, segments, tot, ntiles, nsup, per_core


def _wrap_idx(local_idx, tot):
    """Pack segment-relative int16 indices into the [128, tot//16] wrapped
    layout dma_gather expects (16-partition wrap, replicated 8x)."""
    a16 = local_idx.reshape(-1, 16).T.astype(np.int16)  # [16, tot//16]
    return np.tile(a16, (8, 1))


def build_program(ntiles, nsup, segments, b2val, reps=1, mode="full", nqueues=4):
    import concourse.bass as bass
    import concourse.mybir as mybir
    from concourse import bacc
    from concourse.tile import TileContext

    nc = bacc.Bacc(None, target_bir_lowering=False, debug=False,
                   num_swdge_queues=nqueues,
                   dynamic_dma_scratch_size=65536)
    tot = segments[-1][1] + segments[-1][2]

    z16 = nc.declare_dram_parameter("z16", [N_NODES, H], mybir.dt.float16, isOutput=False)
    idx_r = nc.declare_dram_parameter("idx_r", [128, tot // 16], mybir.dt.int16, isOutput=False)
    idx_c = nc.declare_dram_parameter("idx_c", [128, tot // 16], mybir.dt.int16, isOutput=False)
    w1aT = nc.declare_dram_parameter("w1aT", [128, 128], mybir.dt.float16, isOutput=False)
    w1bT = nc.declare_dram_parameter("w1bT", [128, 128], mybir.dt.float16, isOutput=False)
    b1d = nc.declare_dram_parameter("b1d", [128, 1], mybir.dt.float32, isOutput=False)
    w2e = nc.declare_dram_parameter("w2e", [128, 32 * 32], mybir.dt.float16, isOutput=False)
    outd = nc.declare_dram_parameter("out", [nsup, 128, TILE], mybir.dt.float32, isOutput=True)

    with TileContext(nc) as tc:
        with (
            tc.tile_pool(name="const", bufs=1) as cpool,
            tc.tile_pool(name="gath", bufs=3) as gpool,
            tc.tile_pool(name="idxp", bufs=3) as ipool,
            tc.tile_pool(name="yp", bufs=3) as ypool,
            tc.tile_pool(name="op", bufs=2) as opool,
            tc.tile_pool(name="upsum", bufs=3, space="PSUM") as upp,
            tc.tile_pool(name="opsum", bufs=2, space="PSUM") as opp,
        ):
            w1a_t = cpool.tile([128, 128], mybir.dt.float16, tag="w1a")
            nc.sync.dma_start(out=w1a_t[:], in_=w1aT[:])
            w1b_t = cpool.tile([128, 128], mybir.dt.float16, tag="w1b")
            nc.sync.dma_start(out=w1b_t[:], in_=w1bT[:])
            b1_t = cpool.tile([128, 1], mybir.dt.float32, tag="b1")
            nc.sync.dma_start(out=b1_t[:], in_=b1d[:])
            w2e_t = cpool.tile([128, 32 * 32], mybir.dt.float16, tag="w2e")
            nc.sync.dma_start(out=w2e_t[:], in_=w2e[:])

            def body(_=None):
                T = 0
                out_ps = None
                for (g, n, off) in segments:
                    base_r = (g // NB) * BWIN
                    base_c = (g % NB) * BWIN
                    win_r = min(BWIN, N_NODES - base_r)
                    win_c = min(BWIN, N_NODES - base_c)
                    o16 = off // 16
                    n16 = n // 16
                    ir_t = ipool.tile([128, n16], mybir.dt.int16, tag="ir")
                    nc.sync.dma_start(out=ir_t[:], in_=idx_r[:, o16:o16 + n16])
                    ic_t = ipool.tile([128, n16], mybir.dt.int16, tag="ic")
                    nc.sync.dma_start(out=ic_t[:], in_=idx_c[:, o16:o16 + n16])
                    zr = gpool.tile([128, 1, n], mybir.dt.float16, tag="zr")
                    zc = gpool.tile([128, 1, n], mybir.dt.float16, tag="zc")
                    if mode == "seqload":
                        zz = z16[:99968, :].rearrange("(b a) h -> b (a h)", b=128)
                        nc.sync.dma_start(out=zr[:, 0, :], in_=zz[:, 0:n])
                        nc.sync.dma_start(out=zc[:, 0, :], in_=zz[:, n:2 * n])
                    else:
                        # Both gathers on SWDGE queue 3: concurrent multi-queue
                        # SWDGE corrupts (shared xbar state), but higher queue
                        # index activates more Q7 desc-gen channels.
                        nc.gpsimd.dma_gather(zr[:], z16[base_r:base_r + win_r, :], ir_t[:],
                                             n, n, H, transpose=True, single_packet=False,
                                             queue_num=nqueues - 1)
                        nc.gpsimd.dma_gather(zc[:], z16[base_c:base_c + win_c, :], ic_t[:],
                                             n, n, H, transpose=True, single_packet=False,
                                             queue_num=nqueues - 1)
                    if mode in ("gather",):
                        continue
                    for t in range(n // TILE):
                        S, pos = T // SUP, T % SUP
                        strip, k = pos // 32, pos % 32
                        if pos == 0:
                            out_ps = opp.tile([128, TILE], mybir.dt.float32, tag="ops")
                        sl = slice(t * TILE, (t + 1) * TILE)
                        u_ps = upp.tile([128, TILE], mybir.dt.float32, tag="u")
                        nc.tensor.matmul(u_ps[:], w1a_t[:], zr[:, 0, sl], start=True, stop=False)
                        nc.tensor.matmul(u_ps[:], w1b_t[:], zc[:, 0, sl], start=False, stop=True)
                        y = ypool.tile([128, TILE], mybir.dt.float16, tag="y")
                        nc.scalar.activation(y[:], u_ps[:], mybir.ActivationFunctionType.Lrelu,
                                             bias=b1_t[:], scale=1.0, alpha=0.01)
                        last_in_strip = (k == 31) or (T == ntiles - 1)
                        nc.tensor.matmul(
                            out_ps[32 * strip:32 * (strip + 1), :],
                            w2e_t[:, 32 * k:32 * (k + 1)],
                            y[:],
                            start=(k == 0), stop=last_in_strip,
                            tile_position=(0, 32 * strip),
                        )
                        T += 1
                        if pos == SUP - 1 or T == ntiles:
                            rows = 32 * (strip + 1)
                            o_sb = opool.tile([128, TILE], mybir.dt.float32, tag="osb")
                            nc.vector.tensor_scalar_add(o_sb[:rows, :], out_ps[:rows, :], float(b2val))
                            nc.sync.dma_start(out=outd[S, 0:rows, :], in_=o_sb[:rows, :])

            if reps == 1:
                body()
            else:
                with tc.For_i(0, reps, 1) as _i:
                    body(_i)

    nc.compile()
    return nc


def prepare_inputs(z, edge_label_index, W1, b1, W2):
    z16 = np.asarray(z).astype(np.float16)
    eli = np.asarray(edge_label_index)
    row = eli[0].astype(np.int64)
    col = eli[1].astype(np.int64)
    caps, segments, tot, ntiles, nsup, per_core = _plan(row, col)

    W1 = np.asarray(W1, np.float32)
    w1aT16 = np.ascontiguousarray(W1[:, :H].T).astype(np.float16)
    w1bT16 = np.ascontiguousarray(W1[:, H:].T).astype(np.float16)
    b1_col = np.asarray(b1, np.float32).reshape(128, 1)
    w2_16 = np.asarray(W2, np.float32)[0].astype(np.float16)
    w2e_np = np.zeros((128, 32 * 32), np.float16)
    for k in range(32):
        w2e_np[:, 32 * k + k] = w2_16

    group_start = np.zeros(NB * NB, np.int64)
    group_start[1:] = np.cumsum(caps)[:-1]

    in_maps = []
    scatter = []  # (order, valid_positions) per core
    for c in range(N_CORES):
        order, r_s, c_s, gs = per_core[c]
        # padded local indices, default 0 (gathers window base, discarded)
        lr = np.zeros(tot, np.int16)
        lc = np.zeros(tot, np.int16)
        sizes = np.bincount(gs, minlength=NB * NB)
        valid_pos = np.empty(EDGES_PER_CORE, np.int64)
        cur = 0
        for g in range(NB * NB):
            sgz = int(sizes[g])
            if sgz == 0:
                continue
            pos = group_start[g] + np.arange(sgz)
            lr[pos] = (r_s[cur:cur + sgz] - (g // NB) * BWIN).astype(np.int16)
            lc[pos] = (c_s[cur:cur + sgz] - (g % NB) * BWIN).astype(np.int16)
            valid_pos[cur:cur + sgz] = pos
            cur += sgz
        in_maps.append({
            "z16": z16,
            "idx_r": _wrap_idx(lr, tot),
            "idx_c": _wrap_idx(lc, tot),
            "w1aT": w1aT16, "w1bT": w1bT16,
            "b1d": b1_col, "w2e": w2e_np,
        })
        scatter.append((order, valid_pos))
    return in_maps, scatter, segments, tot, ntiles, nsup


def assemble_output(results, scatter, nsup):
    out = np.empty(E_TOTAL, np.float32)
    for c in range(N_CORES):
        dev = results[c]["out"].reshape(nsup * 128 * TILE)
        order, valid_pos = scatter[c]
        oc = np.empty(EDGES_PER_CORE, np.float32)
        oc[order] = dev[valid_pos]
        out[c * EDGES_PER_CORE:(c + 1) * EDGES_PER_CORE] = oc
    return out


def kernel(z, edge_label_index, W1, b1, W2, b2):
    from concourse.bass_utils import run_bass_kernel_spmd

    in_maps, scatter, segments, tot, ntiles, nsup = prepare_inputs(
        z, edge_label_index, W1, b1, W2)
    b2val = float(np.asarray(b2).reshape(-1)[0])
    nc = build_program(ntiles, nsup, segments, b2val, reps=1)
    res = run_bass_kernel_spmd(nc, in_maps, list(range(N_CORES)))
    return assemble_output(res.results, scatter, nsup)



# revision 17
# speedup vs baseline: 1.0679x; 1.0679x over previous
"""Trainium2 Bass kernel for nn_DNDecoder (GNN edge-MLP decoder).

out[e] = W2 @ LeakyReLU(W1 @ [z[row_e]; z[col_e]] + b1) + b2   for 1.6M edges.

Strategy (8 NeuronCores, edges sharded data-parallel):
  - z is cast to fp16 and replicated on every core; per-edge node features are
    fetched with the GPSIMD transposed dma_gather, which lands z rows as
    *columns* [h=128 partitions, e free] directly in SBUF — the layout matmul
    needs, no on-chip transposes.
  - dma_gather indices are int16 (<32768), so nodes are split into 4 windows
    of 32768; each core's edges are sorted into the 16 (row-window,
    col-window) groups. Group capacities are shared across cores so one SPMD
    program serves all 8 cores. Output is un-permuted on the host.
  - Both gathers ride SWDGE queue 3 (nqueues=4): concurrent use of multiple
    SWDGE queues corrupts data / crashes (shared transpose-xbar state), but a
    higher queue index activates more Q7 desc-gen channels, and a 64KB
    descriptor ring (4096 descs/engine vs default 1024) lets one 8192-idx
    call's desc-gen overlap the previous call's drain.
  - Per 512-edge tile: PSUM U = W1aT.T@zr + W1bT.T@zc (2 fp16 matmuls),
    one ScalarE activation does bias + LeakyReLU + fp16 cast, then a third
    matmul with W2 embedded at the tile's column of a [128,32] stationary
    accumulates 128 tiles of final dot products into one PSUM bank
    ([128 tiles, 512 lanes]), copied out once per 65536 edges.
"""

import sys

for p in ("/opt/trn_rl_repo", "/opt/pypackages"):
    if p not in sys.path:
        sys.path.append(p)

import numpy as np

N_NODES = 100000
H = 128
E_TOTAL = 1600000
N_CORES = 8
EDGES_PER_CORE = E_TOTAL // N_CORES
BWIN = 32768          # index window (int16 gather limit)
NB = 4                # node windows
TILE = 512            # edges per matmul tile (one PSUM bank)
SUP = 128             # tiles per output supertile (one PSUM bank of results)
GATHER_N = 8192       # max edges per dma_gather call
SORT_KEY = "gr"       # within-group edge order: "g" none, "gr" by row, "gc" by col


def _plan(row, col):
    """Shared-structure plan across cores.

    Returns (caps, segments, NTILES, NSUP) and per-core
    (order, local_r, local_c) where order[i] = original edge position of the
    i-th edge in group-sorted order.
    """
    per_core = []
    sizes = np.zeros((N_CORES, NB * NB), np.int64)
    for c in range(N_CORES):
        r = row[c * EDGES_PER_CORE:(c + 1) * EDGES_PER_CORE]
        cc = col[c * EDGES_PER_CORE:(c + 1) * EDGES_PER_CORE]
        g = (r >> 15) * NB + (cc >> 15)
        # sort by (group, row): ascending row idx within a group gives each
        # DMA engine a quasi-sequential HBM read stream for the zr gather
        if SORT_KEY == "gr":
            key = (g.astype(np.int64) << 17) | r
        elif SORT_KEY == "gc":
            key = (g.astype(np.int64) << 17) | cc
        else:
            key = g
        order = np.argsort(key, kind="stable")
        gs = g[order]
        sizes[c] = np.bincount(g, minlength=NB * NB)
        per_core.append((order, r[order], cc[order], gs))
    caps = ((sizes.max(axis=0) + TILE - 1) // TILE) * TILE
    segments = []  # (group, n, tot_off)
    off = 0
    for g in range(NB * NB):
        rem = int(caps[g])
        while rem > 0:
            n = min(GATHER_N, rem)
            segments.append((g, n, off))
            off += n
            rem -= n
    tot = off
    ntiles = tot // TILE
    nsup = (ntiles + SUP - 1) // SUP
    return caps, segments, tot, ntiles, nsup, per_core


def _wrap_idx(local_idx, tot):
    """Pack segment-relative int16 indices into the [128, tot//16] wrapped
    layout dma_gather expects (16-partition wrap, replicated 8x)."""
    a16 = local_idx.reshape(-1, 16).T.astype(np.int16)  # [16, tot//16]
    return np.tile(a16, (8, 1))


def build_program(ntiles, nsup, segments, b2val, reps=1, mode="full", nqueues=4):
    import concourse.bass as bass
    import concourse.mybir as mybir
    from concourse import bacc
    from concourse.tile import TileContext

    nc = bacc.Bacc(None, target_bir_lowering=False, debug=False,
                   num_swdge_queues=nqueues,
                   dynamic_dma_scratch_size=65536)
    tot = segments[-1][1] + segments[-1][2]

    z16 = nc.declare_dram_parameter("z16", [N_NODES, H], mybir.dt.float16, isOutput=False)
    idx_r = nc.declare_dram_parameter("idx_r", [128, tot // 16], mybir.dt.int16, isOutput=False)
    idx_c = nc.declare_dram_parameter("idx_c", [128, tot // 16], mybir.dt.int16, isOutput=False)
    w1aT = nc.declare_dram_parameter("w1aT", [128, 128], mybir.dt.float16, isOutput=False)
    w1bT = nc.declare_dram_parameter("w1bT", [128, 128], mybir.dt.float16, isOutput=False)
    b1d = nc.declare_dram_parameter("b1d", [128, 1], mybir.dt.float32, isOutput=False)
    w2e = nc.declare_dram_parameter("w2e", [128, 32 * 32], mybir.dt.float16, isOutput=False)
    outd = nc.declare_dram_parameter("out", [nsup, 128, TILE], mybir.dt.float32, isOutput=True)

    with TileContext(nc) as tc:
        with (
            tc.tile_pool(name="const", bufs=1) as cpool,
            tc.tile_pool(name="gath", bufs=2) as gpool,
            tc.tile_pool(name="idxp", bufs=2) as ipool,
            tc.tile_pool(name="yp", bufs=3) as ypool,
            tc.tile_pool(name="op", bufs=2) as opool,
            tc.tile_pool(name="upsum", bufs=3, space="PSUM") as upp,
            tc.tile_pool(name="opsum", bufs=2, space="PSUM") as opp,
        ):
            w1a_t = cpool.tile([128, 128], mybir.dt.float16, tag="w1a")
            nc.sync.dma_start(out=w1a_t[:], in_=w1aT[:])
            w1b_t = cpool.tile([128, 128], mybir.dt.float16, tag="w1b")
            nc.sync.dma_start(out=w1b_t[:], in_=w1bT[:])
            b1_t = cpool.tile([128, 1], mybir.dt.float32, tag="b1")
            nc.sync.dma_start(out=b1_t[:], in_=b1d[:])
            w2e_t = cpool.tile([128, 32 * 32], mybir.dt.float16, tag="w2e")
            nc.sync.dma_start(out=w2e_t[:], in_=w2e[:])

            def body(_=None):
                T = 0
                out_ps = None
                for (g, n, off) in segments:
                    base_r = (g // NB) * BWIN
                    base_c = (g % NB) * BWIN
                    win_r = min(BWIN, N_NODES - base_r)
                    win_c = min(BWIN, N_NODES - base_c)
                    o16 = off // 16
                    n16 = n // 16
                    ir_t = ipool.tile([128, n16], mybir.dt.int16, tag="ir")
                    nc.sync.dma_start(out=ir_t[:], in_=idx_r[:, o16:o16 + n16])
                    ic_t = ipool.tile([128, n16], mybir.dt.int16, tag="ic")
                    nc.sync.dma_start(out=ic_t[:], in_=idx_c[:, o16:o16 + n16])
                    zr = gpool.tile([128, 1, n], mybir.dt.float16, tag="zr")
                    zc = gpool.tile([128, 1, n], mybir.dt.float16, tag="zc")
                    if mode == "seqload":
                        zz = z16[:99968, :].rearrange("(b a) h -> b (a h)", b=128)
                        nc.sync.dma_start(out=zr[:, 0, :], in_=zz[:, 0:n])
                        nc.sync.dma_start(out=zc[:, 0, :], in_=zz[:, n:2 * n])
                    else:
                        # Both gathers on SWDGE queue 3: concurrent multi-queue
                        # SWDGE corrupts (shared xbar state), but higher queue
                        # index activates more Q7 desc-gen channels.
                        nc.gpsimd.dma_gather(zr[:], z16[base_r:base_r + win_r, :], ir_t[:],
                                             n, n, H, transpose=True, single_packet=False,
                                             queue_num=nqueues - 1)
                        nc.gpsimd.dma_gather(zc[:], z16[base_c:base_c + win_c, :], ic_t[:],
                                             n, n, H, transpose=True, single_packet=False,
                                             queue_num=nqueues - 1)
                    if mode in ("gather",):
                        continue
                    for t in range(n // TILE):
                        S, pos = T // SUP, T % SUP
                        strip, k = pos // 32, pos % 32
                        if pos == 0:
                            out_ps = opp.tile([128, TILE], mybir.dt.float32, tag="ops")
                        sl = slice(t * TILE, (t + 1) * TILE)
                        u_ps = upp.tile([128, TILE], mybir.dt.float32, tag="u")
                        nc.tensor.matmul(u_ps[:], w1a_t[:], zr[:, 0, sl], start=True, stop=False)
                        nc.tensor.matmul(u_ps[:], w1b_t[:], zc[:, 0, sl], start=False, stop=True)
                        y = ypool.tile([128, TILE], mybir.dt.float16, tag="y")
                        nc.scalar.activation(y[:], u_ps[:], mybir.ActivationFunctionType.Lrelu,
                                             bias=b1_t[:], scale=1.0, alpha=0.01)
                        last_in_strip = (k == 31) or (T == ntiles - 1)
                        nc.tensor.matmul(
                            out_ps[32 * strip:32 * (strip + 1), :],
                            w2e_t[:, 32 * k:32 * (k + 1)],
                            y[:],
                            start=(k == 0), stop=last_in_strip,
                            tile_position=(0, 32 * strip),
                        )
                        T += 1
                        if pos == SUP - 1 or T == ntiles:
                            rows = 32 * (strip + 1)
                            o_sb = opool.tile([128, TILE], mybir.dt.float32, tag="osb")
                            nc.vector.tensor_scalar_add(o_sb[:rows, :], out_ps[:rows, :], float(b2val))
                            nc.sync.dma_start(out=outd[S, 0:rows, :], in_=o_sb[:rows, :])

            if reps == 1:
                body()
            else:
                with tc.For_i(0, reps, 1) as _i:
                    body(_i)

    nc.compile()
    return nc


def prepare_inputs(z, edge_label_index, W1, b1, W2):
    z16 = np.asarray(z).astype(np.float16)
    eli = np.asarray(edge_label_index)
    row = eli[0].astype(np.int64)
    col = eli[1].astype(np.int64)
    caps, segments, tot, ntiles, nsup, per_core = _plan(row, col)

    W1 = np.asarray(W1, np.float32)
    w1aT16 = np.ascontiguousarray(W1[:, :H].T).astype(np.float16)
    w1bT16 = np.ascontiguousarray(W1[:, H:].T).astype(np.float16)
    b1_col = np.asarray(b1, np.float32).reshape(128, 1)
    w2_16 = np.asarray(W2, np.float32)[0].astype(np.float16)
    w2e_np = np.zeros((128, 32 * 32), np.float16)
    for k in range(32):
        w2e_np[:, 32 * k + k] = w2_16

    group_start = np.zeros(NB * NB, np.int64)
    group_start[1:] = np.cumsum(caps)[:-1]

    in_maps = []
    scatter = []  # (order, valid_positions) per core
    for c in range(N_CORES):
        order, r_s, c_s, gs = per_core[c]
        # padded local indices, default 0 (gathers window base, discarded)
        lr = np.zeros(tot, np.int16)
        lc = np.zeros(tot, np.int16)
        sizes = np.bincount(gs, minlength=NB * NB)
        valid_pos = np.empty(EDGES_PER_CORE, np.int64)
        cur = 0
        for g in range(NB * NB):
            sgz = int(sizes[g])
            if sgz == 0:
                continue
            pos = group_start[g] + np.arange(sgz)
            lr[pos] = (r_s[cur:cur + sgz] - (g // NB) * BWIN).astype(np.int16)
            lc[pos] = (c_s[cur:cur + sgz] - (g % NB) * BWIN).astype(np.int16)
            valid_pos[cur:cur + sgz] = pos
            cur += sgz
        in_maps.append({
            "z16": z16,
            "idx_r": _wrap_idx(lr, tot),
            "idx_c": _wrap_idx(lc, tot),
            "w1aT": w1aT16, "w1bT": w1bT16,
            "b1d": b1_col, "w2e": w2e_np,
        })
        scatter.append((order, valid_pos))
    return in_maps, scatter, segments, tot, ntiles, nsup


def assemble_output(results, scatter, nsup):
    out = np.empty(E_TOTAL, np.float32)
    for c in range(N_CORES):
        dev = results[c]["out"].reshape(nsup * 128 * TILE)
        order, valid_pos = scatter[c]
        oc = np.empty(EDGES_PER_CORE, np.float32)
        oc[order] = dev[valid_pos]
        out[c * EDGES_PER_CORE:(c + 1) * EDGES_PER_CORE] = oc
    return out


def kernel(z, edge_label_index, W1, b1, W2, b2):
    from concourse.bass_utils import run_bass_kernel_spmd

    in_maps, scatter, segments, tot, ntiles, nsup = prepare_inputs(
        z, edge_label_index, W1, b1, W2)
    b2val = float(np.asarray(b2).reshape(-1)[0])
    nc = build_program(ntiles, nsup, segments, b2val, reps=1)
    res = run_bass_kernel_spmd(nc, in_maps, list(range(N_CORES)))
    return assemble_output(res.results, scatter, nsup)

